# revision 24
# baseline (speedup 1.0000x reference)
"""Trainium2 Bass kernel for nn_FCNet (dense MLP, 8 layers).

Network: x[B,P,64] -> Linear(64->1024) -> 7x (ReLU -> Linear(1024->1024))
with B=4, P=2048 (8192 rows total), fp32 reference.

Strategy (8 NeuronCores, data-parallel):
  - Shard the 8192 rows across 8 cores (1024 rows each); replicate weights.
  - Feature-major on-chip layout: h[t] lives as [128 partitions, 8 feature
    chunks, rows] so every layer is PSUM-accumulated matmuls with lhsT =
    W.T tiles (pre-transposed on host) and rhs = h — no on-chip transposes.
  - Matmuls run in fp16 (1 cycle/row on the PE vs 4 for fp32) with fp32
    PSUM accumulation; per-layer rel-RMS error ~3e-4, ~8e-4 end to end.
  - All 7 hidden-layer weights (fp16) stay resident in SBUF (112 KB per
    partition), DMA'd once and prefetched under layer-0 compute.
  - Rows are processed in two blocks of 512 (PSUM bank = 512 fp32), and the
    two blocks are interleaved within each layer so the PE never waits for
    the previous layer's last evacuation.
  - PSUM -> SBUF evacuation is a fused bias+ReLU op (out = max(psum+b, 0),
    written as fp16): hidden layers use one DVE tensor_scalar per group
    (group cadence 1.7us >> evac ~0.66us); layer 0's single-matmul groups
    alternate whole groups between DVE and ACT so neither engine's latency
    paces PSUM-bank recycling; the final layer splits each fp32 evac half
    DVE (add+bypass) / half ACT (Identity+bias, both bit-exact) and issues
    two half-width output DMAs per group, shortening the single-shot tail.
    Prologue DMAs are split so the first matmul starts after ~160KB lands.

Measured on hardware (loop-scaling with device-resident inputs): the chip
clock ramps under load — a cold burst (single-shot regime) runs ~180 us/iter
(~2.6 GHz effective), decaying to ~242-275 us/iter (~1.7-1.9 GHz) after
~15 ms of sustained load. Per-matmul cost is pure streaming (rows x cycle,
zero fixed intercept — verified by 256/512-row and same-vs-rotating-weights
microbenches), i.e. this schedule sits at the fp16 PE streaming floor and
only the clock state varies. fp8 DoubleRow was measured at 2x fp16 FLOPs
(not the cost model's 4x); with the 2e-2 error gate it cannot beat fp16
(pure fp8 errs ~9e-2; the accurate 3-term residual-split scheme needs 1.5x
the instructions and loses). Relative RMS error vs fp32 reference: 8.9e-4.

Follow-up characterization (this session): the PE streaming rate is
strongly DATA-DEPENDENT (power/DVFS): the identical matmul stream runs
~120-160 us/iter on all-zero activations, ~160-200 on constant ones, and
~200-275 on N(0,1) data — the same spread as the full kernel. A matmuls-
only loop on zero data therefore reads ~40-50 us/iter faster than the real
kernel, which is clock state, not schedule stall; evac-strategy A/B probes
(engine splits, slicing, multi-bank ops, whole-group DVE/ACT alternation)
all land within DVFS noise of each other, with this exact schedule the
fastest measured. Isolated-engine microbenches: DVE tensor_scalar
[128,512] PSUM->SBUF = ~670ns, ACT = ~605ns, both far under the per-group
cadence, confirming evacuation is not throughput-bound. bf16 operands
measured identical sustained speed to fp16 (no power/DVFS advantage), so
fp16 is kept for its 4x better precision. TimelineSim: this schedule's
steady-state loop body is PE-gap-free (per-iteration delta = 194.85us vs
the 194.56us 466944-cycle floor at the sim's 2.4GHz); single-shot 205.1us
incl. DMA prologue/tail. DMA cannot read PSUM on this framework (evac
must use DVE/ACT), and matmul output must be fp32 PSUM.
"""

import numpy as np

import concourse.bacc as bacc
import concourse.mybir as mybir
import concourse.tile as tile
from concourse.bass_utils import run_bass_kernel_spmd

P = 128          # partitions
VEC = 1024       # hidden/output width
IN = 64          # input feature dim
K_LAYERS = 7     # hidden Linear layers after the first
N_CORES = 8
ROWS_PER_CORE = 1024
RB = 512         # row block (one PSUM bank of fp32)
NRB = ROWS_PER_CORE // RB
NC_FEAT = VEC // P  # 8 feature chunks

_cached = {}


def _evac_relu(nc, out_ap, ps, bias_ap, alt=False):
    """PSUM -> SBUF fp16, out = relu(psum + bias).

    Default engine is DVE (one tensor_scalar: add then max) — measured to
    keep up with the PE for the 1024x1024 layers, whose groups take 8
    matmuls (~1.7us) per evac (~0.66us). Layer 0's groups are a single
    matmul (~0.21us), so a lone DVE falls 3x behind and the PE stalls on
    PSUM-bank recycling (~2.6us/pass in TimelineSim); layer 0 therefore
    alternates whole groups between DVE and ACT (alt=True -> ACT:
    relu(x + bias), same math; every ACT table contains both relu and
    identity so there is no table-swap cost against the final-layer
    Identity evacs)."""
    if alt:
        nc.scalar.activation(
            out_ap, ps[:],
            mybir.ActivationFunctionType.Relu, bias=bias_ap)
    else:
        nc.vector.tensor_scalar(
            out_ap, ps[:], bias_ap, 0.0,
            mybir.AluOpType.add, mybir.AluOpType.max)


def _evac_final(nc, o_sb, ps, bias_ap):
    """Final layer PSUM -> SBUF, out = psum + bias (no relu), split in
    half across DVE (add + bypass) and ACT (Identity-with-bias). The
    halved per-engine latency releases PSUM banks quickly at the loop
    boundary and lets the two output DMAs start ~0.7us apart, shortening
    the single-shot tail. Output is written as fp16 (upconverted to fp32
    on the host): halves the output DMA traffic (4MB -> 2MB per core per
    pass) and the evac write volume, adding only ~2.8e-4 rel RMS output
    rounding (total ~9.3e-4 vs the 2e-2 gate)."""
    hc = RB // 2
    nc.vector.tensor_scalar(
        o_sb[:, 0:hc], ps[:, 0:hc], bias_ap, 0.0,
        mybir.AluOpType.add, mybir.AluOpType.bypass)
    nc.scalar.activation(
        o_sb[:, hc:], ps[:, hc:],
        mybir.ActivationFunctionType.Identity, bias=bias_ap)


def _build_nc(loop=False, nbody=1):
    """Per-core program: out[1024,1024] (feature-major) = MLP(x shard).

    loop=True adds a `niter` input and wraps the whole layer stack in a
    runtime For_i — used only for hardware timing (the per-iteration slope
    isolates on-device time from host/dispatch overhead). nbody>1 statically
    unrolls the body instead (used only by simcheck.py to expose
    iteration-boundary stalls in TimelineSim).
    """
    nc = bacc.Bacc("TRN2", target_bir_lowering=False, debug=False)
    f16, f32 = mybir.dt.float16, mybir.dt.float32

    xT = nc.dram_tensor("xT", [P, ROWS_PER_CORE], f16, kind="ExternalInput")
    w0T = nc.dram_tensor("w0T", [P, VEC], f16, kind="ExternalInput")
    whT = nc.dram_tensor("whT", [K_LAYERS, VEC, VEC], f16, kind="ExternalInput")
    bias = nc.dram_tensor("bias", [P, (K_LAYERS + 1) * NC_FEAT], f32,
                          kind="ExternalInput")
    if loop:
        niter = nc.dram_tensor("niter", [1, 1], mybir.dt.uint32,
                               kind="ExternalInput")
    out = nc.dram_tensor("out", [VEC, ROWS_PER_CORE], f16, kind="ExternalOutput")
    out3 = out.rearrange("(oc p) r -> p oc r", p=P)

    with tile.TileContext(nc) as tc:
        with (
            tc.tile_pool(name="wpool", bufs=1) as wpool,
            tc.tile_pool(name="hpool", bufs=1) as hpool,
            tc.tile_pool(name="opool", bufs=8) as opool,
            tc.tile_pool(name="psum", bufs=8, space="PSUM") as psum,
        ):
            x_sb = wpool.tile([P, ROWS_PER_CORE], f16, tag="x")
            w0_sb = wpool.tile([P, VEC], f16, tag="w0")
            b_sb = wpool.tile([P, (K_LAYERS + 1) * NC_FEAT], f32, tag="b")
            wh_sb = wpool.tile([P, K_LAYERS * NC_FEAT, VEC], f16, tag="wh")

            # Prologue DMAs split so the first matmul (needs w0 chunk 0 +
            # x row-block 0) starts as soon as ~160KB has landed, not after
            # the full 576KB of x+w0+bias. (Finer chunking was tried and is
            # WORSE in TimelineSim: ~1us fixed overhead per DMA dominates.)
            nc.sync.dma_start(w0_sb[:, 0:P], w0T[:, 0:P])
            nc.sync.dma_start(x_sb[:, 0:RB], xT[:, 0:RB])
            nc.sync.dma_start(b_sb[:], bias[:])
            nc.sync.dma_start(w0_sb[:, P:], w0T[:, P:])
            nc.sync.dma_start(x_sb[:, RB:], xT[:, RB:])
            for l in range(K_LAYERS):
                for kc in range(NC_FEAT):
                    nc.sync.dma_start(
                        wh_sb[:, l * NC_FEAT + kc, :],
                        whT[l, kc * P:(kc + 1) * P, :],
                    )

            # ping-pong activation buffers, one pair per row block
            h = [[hpool.tile([P, NC_FEAT, RB], f16, tag=f"h_{rb}_{s}",
                             name=f"h_{rb}_{s}")
                  for s in range(2)] for rb in range(NRB)]

            def emit_body():
                # layer 0: contract IN (padded to 128), one matmul per (rb, oc)
                for rb in range(NRB):
                    for oc in range(NC_FEAT):
                        ps = psum.tile([P, RB], f32, tag="ps", name="ps0")
                        nc.tensor.matmul(
                            ps[:],
                            w0_sb[:, oc * P:(oc + 1) * P],
                            x_sb[:, rb * RB:(rb + 1) * RB],
                            start=True, stop=True,
                        )
                        _evac_relu(nc, h[rb][0][:, oc, :], ps, b_sb[:, oc:oc + 1],
                                   alt=(oc % 2 == 1))

                # hidden layers
                for j in range(1, K_LAYERS + 1):
                    wbase = (j - 1) * NC_FEAT
                    bcol = j * NC_FEAT
                    src, dst = (j + 1) % 2, j % 2
                    for rb in range(NRB):
                        h_in = h[rb][src]
                        for oc in range(NC_FEAT):
                            ps = psum.tile([P, RB], f32, tag="ps", name="ps")
                            for kc in range(NC_FEAT):
                                nc.tensor.matmul(
                                    ps[:],
                                    wh_sb[:, wbase + kc, oc * P:(oc + 1) * P],
                                    h_in[:, kc, :],
                                    start=(kc == 0), stop=(kc == NC_FEAT - 1),
                                )
                            if j < K_LAYERS:
                                _evac_relu(nc, h[rb][dst][:, oc, :], ps,
                                           b_sb[:, bcol + oc:bcol + oc + 1])
                            else:
                                o_sb = opool.tile([P, RB], f16, tag="o",
                                                  name="o_sb")
                                _evac_final(nc, o_sb, ps,
                                            b_sb[:, bcol + oc:bcol + oc + 1])
                                hc = RB // 2
                                r0 = rb * RB
                                nc.sync.dma_start(
                                    out3[:, oc, r0:r0 + hc], o_sb[:, 0:hc])
                                nc.sync.dma_start(
                                    out3[:, oc, r0 + hc:r0 + RB], o_sb[:, hc:])

            if loop:
                n_sb = wpool.tile([1, 1], mybir.dt.uint32, tag="niter")
                nc.sync.dma_start(n_sb[:], niter[:])
                n_rv = nc.values_load(n_sb[0:1, 0:1], max_val=1 << 20,
                                      skip_runtime_bounds_check=True)
                with tc.For_i(0, n_rv, 1):
                    emit_body()
            else:
                for _ in range(nbody):
                    emit_body()
    nc.compile()
    return nc


def _get_nc(loop=False):
    key = "nc_loop" if loop else "nc"
    if key not in _cached:
        _cached[key] = _build_nc(loop=loop)
    return _cached[key]


def build_in_maps(x, W0, b0, Wh, bh):
    x = np.asarray(x, dtype=np.float32)
    W0 = np.asarray(W0, dtype=np.float32)
    b0 = np.asarray(b0, dtype=np.float32)
    Wh = np.asarray(Wh, dtype=np.float32)
    bh = np.asarray(bh, dtype=np.float32)
    B, Pp, _ = x.shape
    rows = B * Pp
    per = rows // N_CORES

    xf = x.reshape(rows, IN)
    w0T = np.zeros((P, VEC), dtype=np.float16)
    w0T[:IN] = W0.T.astype(np.float16)
    whT = np.ascontiguousarray(Wh.transpose(0, 2, 1)).astype(np.float16)

    bias = np.zeros((P, (K_LAYERS + 1) * NC_FEAT), dtype=np.float32)
    bias[:, :NC_FEAT] = b0.reshape(NC_FEAT, P).T
    for l in range(K_LAYERS):
        bias[:, (l + 1) * NC_FEAT:(l + 2) * NC_FEAT] = bh[l].reshape(NC_FEAT, P).T

    in_maps = []
    for c in range(N_CORES):
        xT = np.zeros((P, per), dtype=np.float16)
        xT[:IN] = xf[c * per:(c + 1) * per].T.astype(np.float16)
        in_maps.append({"xT": xT, "w0T": w0T, "whT": whT, "bias": bias})
    return in_maps


def kernel(x, W0, b0, Wh, bh):
    B, Pp, _ = np.asarray(x).shape
    in_maps = build_in_maps(x, W0, b0, Wh, bh)
    res = run_bass_kernel_spmd(_get_nc(), in_maps, list(range(N_CORES)))
    outs = [res.results[c]["out"].T for c in range(N_CORES)]  # [rows, VEC]
    return np.concatenate(outs, axis=0).reshape(B, Pp, VEC).astype(np.float32)

